# revision 16
# baseline (speedup 1.0000x reference)
"""Trainium2 Bass kernel for nn_ContinuumMemoryCell (scatter_memory).

Data-parallel over batch B across 8 NeuronCores. Device does the three
B-sized matmuls (error = x @ (V_w.T - M), y_pred = x @ M, and the Hebbian
partial dp_i = error_i.T @ x_i) plus the fused elementwise output
out = y_pred + mix * error. Everything O(D*H) or smaller (gate vectors,
sigmoid means, the final new_M AXPY, shard stitching) happens on host.
"""

import sys

if "/opt/trn_rl_repo" not in sys.path:
    sys.path.insert(0, "/opt/trn_rl_repo")

import numpy as np
import ml_dtypes

B, D, H = 16384, 1024, 1024
NCORES = 8
BL = B // NCORES          # 2048 batch rows per core
P = 128                   # partitions
NB = BL // P              # 16 b-tiles per core
NK = D // P               # 8 k-tiles (contraction over d)
NH = H // P               # 8 h-tiles (dp output rows)
FD = 512                  # matmul moving free-dim (one PSUM bank of f32)

_CACHE = {}


def _build():
    """Build + compile the SPMD Bass program (once per process)."""
    if "nc" in _CACHE:
        return _CACHE["nc"]

    import concourse.bacc as bacc
    import concourse.mybir as mybir
    import concourse.tile as tile

    bf16 = mybir.dt.bfloat16
    f32 = mybir.dt.float32

    nc = bacc.Bacc("TRN2", target_bir_lowering=False, debug=False,
                   num_devices=NCORES)

    xT_d = nc.dram_tensor("xT", [D, BL], bf16, kind="ExternalInput")
    xn_d = nc.dram_tensor("xn", [BL, D], bf16, kind="ExternalInput")
    wc_d = nc.dram_tensor("wc", [D, H + D], bf16, kind="ExternalInput")
    mx_d = nc.dram_tensor("mx", [P, NB], f32, kind="ExternalInput")
    out_d = nc.dram_tensor("out", [BL, H], f32, kind="ExternalOutput")
    dp_d = nc.dram_tensor("dp", [H, D], f32, kind="ExternalOutput")

    with tile.TileContext(nc) as tc:
        with (
            tc.tile_pool(name="big", bufs=1) as big,
            tc.tile_pool(name="work", bufs=4) as work,
            tc.tile_pool(name="ps", bufs=4, space="PSUM") as ps,
        ):
            # Resident SBUF tensors, one tile per 128-row chunk so Tile
            # tracks DMA->matmul deps at chunk granularity.
            xT_sb = [big.tile([P, BL], bf16, tag=f"xT{k}", name=f"xT{k}")
                     for k in range(NK)]
            wc_sb = [big.tile([P, H + D], bf16, tag=f"wc{k}", name=f"wc{k}")
                     for k in range(NK)]
            xn_sb = [big.tile([P, D], bf16, tag=f"xn{i}", name=f"xn{i}")
                     for i in range(NB)]
            err_sb = [big.tile([P, H], bf16, tag=f"err{i}", name=f"err{i}")
                      for i in range(NB)]
            mx_sb = big.tile([P, NB], f32, tag="mx", name="mx")
            warm = big.tile([P, FD], bf16, tag="warm", name="warm")

            # Warm the PE HAM clock gate while input DMAs stream.
            pw = ps.tile([P, FD], f32, tag="acc", name="pw")
            nc.vector.memset(warm[:], 0.0)
            for _ in range(14):
                nc.tensor.matmul(pw[:], warm[:, 0:P], warm[:], start=True,
                                 stop=True)

            # Input loads: triggers round-robined over the three DMA-capable
            # engines in need order, so chunk k of xT and wc land together
            # and in ascending k.
            load_eng = [nc.sync, nc.gpsimd, nc.scalar]
            chunks = []
            for k in range(NK):
                chunks.append((xT_sb[k], xT_d[k * P:(k + 1) * P, :]))
                chunks.append((wc_sb[k], wc_d[k * P:(k + 1) * P, :]))
            for i in range(NB):
                chunks.append((xn_sb[i], xn_d[i * P:(i + 1) * P, :]))
            for j, (dst, src) in enumerate(chunks):
                load_eng[j % 3].dma_start(dst[:], src)
            nc.gpsimd.dma_start(mx_sb[:], mx_d[:])

            # Phase 1: per b-tile, err = x @ We and y = x @ M with the same
            # stationary xT tile; fused epilogue out = err*mix + y.
            for i in range(NB):
                pe = ps.tile([P, H], f32, tag="acc", name=f"pe{i}")
                py = ps.tile([P, D], f32, tag="acc", name=f"py{i}")
                for k in range(NK):
                    lhs = xT_sb[k][:, i * P:(i + 1) * P]
                    st, sp = (k == 0), (k == NK - 1)
                    for h2 in range(2):
                        nc.tensor.matmul(pe[:, h2 * FD:(h2 + 1) * FD], lhs,
                                         wc_sb[k][:, h2 * FD:(h2 + 1) * FD],
                                         start=st, stop=sp)
                    for h2 in range(2):
                        nc.tensor.matmul(py[:, h2 * FD:(h2 + 1) * FD], lhs,
                                         wc_sb[k][:, H + h2 * FD:H + (h2 + 1) * FD],
                                         start=st, stop=sp)
                nc.any.tensor_copy(err_sb[i][:], pe[:])
                for h2 in range(2):
                    o = work.tile([P, FD], f32, tag="o", name=f"o{i}_{h2}")
                    nc.vector.scalar_tensor_tensor(
                        o[:], err_sb[i][:, h2 * FD:(h2 + 1) * FD],
                        mx_sb[:, i:i + 1],
                        py[:, h2 * FD:(h2 + 1) * FD],
                        mybir.AluOpType.mult, mybir.AluOpType.add)
                    load_eng[(2 * i + h2) % 3].dma_start(
                        out_d[i * P:(i + 1) * P, h2 * FD:(h2 + 1) * FD], o[:])

            # Phase 2: dp[h-tile] = sum_b err[b, h-tile].T @ x[b, :]
            for t in range(NH):
                pd = ps.tile([P, D], f32, tag="acc", name=f"pd{t}")
                for i in range(NB):
                    lhs = err_sb[i][:, t * P:(t + 1) * P]
                    st, sp = (i == 0), (i == NB - 1)
                    for h2 in range(2):
                        nc.tensor.matmul(pd[:, h2 * FD:(h2 + 1) * FD], lhs,
                                         xn_sb[i][:, h2 * FD:(h2 + 1) * FD],
                                         start=st, stop=sp)
                dpt = work.tile([P, D], f32, tag="dpt", name=f"dpt{t}")
                for h2 in range(2):
                    nc.any.tensor_copy(dpt[:, h2 * FD:(h2 + 1) * FD],
                                       pd[:, h2 * FD:(h2 + 1) * FD])
                    nc.gpsimd.dma_start(
                        dp_d[t * P:(t + 1) * P, h2 * FD:(h2 + 1) * FD],
                        dpt[:, h2 * FD:(h2 + 1) * FD])

    nc.compile()
    _CACHE["nc"] = nc
    return nc


def _prepare(inputs):
    """Host-side preprocessing: shard + dtype-convert + gate math."""
    x = np.asarray(inputs["x"], np.float32)
    V_w = np.asarray(inputs["V_w"], np.float32)
    M = np.asarray(inputs["M"], np.float32)
    fg_w = np.asarray(inputs["fg_w"], np.float32)
    fg_b = np.asarray(inputs["fg_b"], np.float32)
    ug_w = np.asarray(inputs["ug_w"], np.float32)
    ug_b = np.asarray(inputs["ug_b"], np.float32)
    sm_w = np.asarray(inputs["sm_w"], np.float32)
    sm_b = np.asarray(inputs["sm_b"], np.float32)

    bf16 = ml_dtypes.bfloat16
    VT = V_w.T                                # [D, H]
    wc = np.concatenate([VT - M, M], axis=1).astype(bf16)   # [D, H+D]

    # v @ a == x @ (V_w.T @ a): collapse each gate to one D-vector on x.
    c_f = VT @ fg_w[0, :H] + fg_w[0, H:]
    c_u = VT @ ug_w[0, :H] + ug_w[0, H:]
    c_m = VT @ sm_w[0]
    logits = x @ np.stack([c_f, c_u, c_m], axis=1)       # [B, 3]
    sig = 1.0 / (1.0 + np.exp(-(logits + np.array([fg_b[0], ug_b[0], sm_b[0]]))))
    fmean = float(sig[:, 0].mean())
    umean = float(sig[:, 1].mean())
    mix = sig[:, 2].astype(np.float32)                   # [B]

    xb = x.astype(bf16)
    in_maps = []
    for i in range(NCORES):
        s = slice(i * BL, (i + 1) * BL)
        in_maps.append({
            "xT": np.ascontiguousarray(xb[s].T),
            "xn": np.ascontiguousarray(xb[s]),
            "wc": wc,
            "mx": np.ascontiguousarray(mix[s].reshape(NB, P).T),
        })
    return in_maps, M, fmean, umean


def _finish(results, M, fmean, umean):
    out = np.concatenate([results[i]["out"] for i in range(NCORES)], axis=0)
    delta = results[0]["dp"].astype(np.float64)
    for i in range(1, NCORES):
        delta += results[i]["dp"]
    delta_mean = (delta / B).astype(np.float32)
    new_M = fmean * M + umean * 0.1 * delta_mean
    return out.astype(np.float32), new_M.astype(np.float32)


def _run(inputs, trace=False, trace_kwargs=None):
    from concourse.bass_utils import run_bass_kernel_spmd

    nc = _build()
    in_maps, M, fmean, umean = _prepare(inputs)
    res = run_bass_kernel_spmd(nc, in_maps, core_ids=list(range(NCORES)),
                               trace=trace, **(trace_kwargs or {}))
    return _finish(res.results, M, fmean, umean), res


def kernel(**inputs):
    (out, new_M), _ = _run(inputs)
    return out, new_M


# revision 19
# speedup vs baseline: 1.0391x; 1.0391x over previous
"""Trainium2 Bass kernel for nn_ContinuumMemoryCell (scatter_memory).

Data-parallel over batch B across 8 NeuronCores. Device does the three
B-sized matmuls (error = x @ (V_w.T - M), y_pred = x @ M, and the Hebbian
partial dp_i = error_i.T @ x_i) plus the fused elementwise output
out = y_pred + mix * error. Everything O(D*H) or smaller (gate vectors,
sigmoid means, the final new_M AXPY, shard stitching) happens on host.
"""

import sys

if "/opt/trn_rl_repo" not in sys.path:
    sys.path.insert(0, "/opt/trn_rl_repo")

import numpy as np
import ml_dtypes

B, D, H = 16384, 1024, 1024
NCORES = 8
BL = B // NCORES          # 2048 batch rows per core
P = 128                   # partitions
NB = BL // P              # 16 b-tiles per core
NK = D // P               # 8 k-tiles (contraction over d)
NH = H // P               # 8 h-tiles (dp output rows)
FD = 512                  # matmul moving free-dim (one PSUM bank of f32)

_CACHE = {}


def _build():
    """Build + compile the SPMD Bass program (once per process)."""
    if "nc" in _CACHE:
        return _CACHE["nc"]

    import concourse.bacc as bacc
    import concourse.mybir as mybir
    import concourse.tile as tile

    bf16 = mybir.dt.bfloat16
    f32 = mybir.dt.float32

    nc = bacc.Bacc("TRN2", target_bir_lowering=False, debug=False,
                   num_devices=NCORES)

    xT_d = nc.dram_tensor("xT", [D, BL], bf16, kind="ExternalInput")
    xn_d = nc.dram_tensor("xn", [BL, D], bf16, kind="ExternalInput")
    we_d = nc.dram_tensor("we", [D, H], bf16, kind="ExternalInput")
    mm_d = nc.dram_tensor("mm", [D, D], bf16, kind="ExternalInput")
    mx_d = nc.dram_tensor("mx", [P, NB], f32, kind="ExternalInput")
    out_d = nc.dram_tensor("out", [BL, H], f32, kind="ExternalOutput")
    dp_d = nc.dram_tensor("dp", [H, D], f32, kind="ExternalOutput")

    with tile.TileContext(nc) as tc:
        with (
            tc.tile_pool(name="big", bufs=1) as big,
            tc.tile_pool(name="work", bufs=4) as work,
            tc.tile_pool(name="ps", bufs=4, space="PSUM") as ps,
        ):
            # Resident SBUF tensors, one tile per 128-row chunk so Tile
            # tracks DMA->matmul deps at chunk granularity.
            xT_sb = [big.tile([P, BL], bf16, tag=f"xT{k}", name=f"xT{k}")
                     for k in range(NK)]
            we_sb = [big.tile([P, H], bf16, tag=f"we{k}", name=f"we{k}")
                     for k in range(NK)]
            mm_sb = [big.tile([P, D], bf16, tag=f"mm{k}", name=f"mm{k}")
                     for k in range(NK)]
            xn_sb = [big.tile([P, D], bf16, tag=f"xn{i}", name=f"xn{i}")
                     for i in range(NB)]
            err_sb = [big.tile([P, H], bf16, tag=f"err{i}", name=f"err{i}")
                      for i in range(NB)]
            mx_sb = big.tile([P, NB], f32, tag="mx", name="mx")

            # Input loads: triggers round-robined over the three DMA-capable
            # engines in need order: the err pass streams xT+we; mm and xn
            # follow (resident long before their passes start). gpsimd's
            # ring additionally carries the out/dp writes.
            load_eng = [nc.sync, nc.gpsimd, nc.scalar]
            chunks = []
            for k in range(NK):
                chunks.append((xT_sb[k], xT_d[k * P:(k + 1) * P, :]))
                chunks.append((we_sb[k], we_d[k * P:(k + 1) * P, :]))
            for j, (dst, src_) in enumerate(chunks):
                load_eng[j % 3].dma_start(dst[:], src_)
            nc.gpsimd.dma_start(mx_sb[:], mx_d[:])
            xn_eng = [nc.sync, nc.scalar]
            for k in range(NK):
                xn_eng[k % 2].dma_start(mm_sb[k][:], mm_d[k * P:(k + 1) * P, :])
            for i in range(NB):
                xn_eng[i % 2].dma_start(xn_sb[i][:], xn_d[i * P:(i + 1) * P, :])

            # Phase 1a: err = x @ (V_w.T - M), stored bf16 in SBUF.
            for i in range(NB):
                pe = ps.tile([P, H], f32, tag="acc", name=f"pe{i}")
                for k in range(NK):
                    lhs = xT_sb[k][:, i * P:(i + 1) * P]
                    st, sp = (k == 0), (k == NK - 1)
                    for h2 in range(2):
                        nc.tensor.matmul(pe[:, h2 * FD:(h2 + 1) * FD], lhs,
                                         we_sb[k][:, h2 * FD:(h2 + 1) * FD],
                                         start=st, stop=sp)
                nc.any.tensor_copy(err_sb[i][:], pe[:])

            # Phase 1b: y = x @ M; fused epilogue out = err*mix + y.
            for i in range(NB):
                py = ps.tile([P, D], f32, tag="acc", name=f"py{i}")
                for k in range(NK):
                    lhs = xT_sb[k][:, i * P:(i + 1) * P]
                    st, sp = (k == 0), (k == NK - 1)
                    for h2 in range(2):
                        nc.tensor.matmul(py[:, h2 * FD:(h2 + 1) * FD], lhs,
                                         mm_sb[k][:, h2 * FD:(h2 + 1) * FD],
                                         start=st, stop=sp)
                for h2 in range(2):
                    o = work.tile([P, FD], f32, tag="o", name=f"o{i}_{h2}",
                                  bufs=8)
                    nc.vector.scalar_tensor_tensor(
                        o[:], err_sb[i][:, h2 * FD:(h2 + 1) * FD],
                        mx_sb[:, i:i + 1],
                        py[:, h2 * FD:(h2 + 1) * FD],
                        mybir.AluOpType.mult, mybir.AluOpType.add)
                    nc.gpsimd.dma_start(
                        out_d[i * P:(i + 1) * P, h2 * FD:(h2 + 1) * FD], o[:])

            # Phase 2: dp[h-tile] = sum_b err[b, h-tile].T @ x[b, :]
            for t in range(NH):
                pd = ps.tile([P, D], f32, tag="acc", name=f"pd{t}")
                for i in range(NB):
                    lhs = err_sb[i][:, t * P:(t + 1) * P]
                    st, sp = (i == 0), (i == NB - 1)
                    for h2 in range(2):
                        nc.tensor.matmul(pd[:, h2 * FD:(h2 + 1) * FD], lhs,
                                         xn_sb[i][:, h2 * FD:(h2 + 1) * FD],
                                         start=st, stop=sp)
                dpt = work.tile([P, D], f32, tag="dpt", name=f"dpt{t}")
                for h2 in range(2):
                    nc.any.tensor_copy(dpt[:, h2 * FD:(h2 + 1) * FD],
                                       pd[:, h2 * FD:(h2 + 1) * FD])
                    nc.gpsimd.dma_start(
                        dp_d[t * P:(t + 1) * P, h2 * FD:(h2 + 1) * FD],
                        dpt[:, h2 * FD:(h2 + 1) * FD])

    nc.compile()
    _CACHE["nc"] = nc
    return nc


def _prepare(inputs):
    """Host-side preprocessing: shard + dtype-convert + gate math."""
    x = np.asarray(inputs["x"], np.float32)
    V_w = np.asarray(inputs["V_w"], np.float32)
    M = np.asarray(inputs["M"], np.float32)
    fg_w = np.asarray(inputs["fg_w"], np.float32)
    fg_b = np.asarray(inputs["fg_b"], np.float32)
    ug_w = np.asarray(inputs["ug_w"], np.float32)
    ug_b = np.asarray(inputs["ug_b"], np.float32)
    sm_w = np.asarray(inputs["sm_w"], np.float32)
    sm_b = np.asarray(inputs["sm_b"], np.float32)

    bf16 = ml_dtypes.bfloat16
    VT = V_w.T                                # [D, H]
    we = np.ascontiguousarray(VT - M).astype(bf16)
    mm = np.ascontiguousarray(M).astype(bf16)

    # v @ a == x @ (V_w.T @ a): collapse each gate to one D-vector on x.
    c_f = VT @ fg_w[0, :H] + fg_w[0, H:]
    c_u = VT @ ug_w[0, :H] + ug_w[0, H:]
    c_m = VT @ sm_w[0]
    logits = x @ np.stack([c_f, c_u, c_m], axis=1)       # [B, 3]
    sig = 1.0 / (1.0 + np.exp(-(logits + np.array([fg_b[0], ug_b[0], sm_b[0]]))))
    fmean = float(sig[:, 0].mean())
    umean = float(sig[:, 1].mean())
    mix = sig[:, 2].astype(np.float32)                   # [B]

    xb = x.astype(bf16)
    in_maps = []
    for i in range(NCORES):
        s = slice(i * BL, (i + 1) * BL)
        in_maps.append({
            "xT": np.ascontiguousarray(xb[s].T),
            "xn": np.ascontiguousarray(xb[s]),
            "we": we,
            "mm": mm,
            "mx": np.ascontiguousarray(mix[s].reshape(NB, P).T),
        })
    return in_maps, M, fmean, umean


def _finish(results, M, fmean, umean):
    out = np.concatenate([results[i]["out"] for i in range(NCORES)], axis=0)
    delta = results[0]["dp"].astype(np.float64)
    for i in range(1, NCORES):
        delta += results[i]["dp"]
    delta_mean = (delta / B).astype(np.float32)
    new_M = fmean * M + umean * 0.1 * delta_mean
    return out.astype(np.float32), new_M.astype(np.float32)


def _run(inputs, trace=False, trace_kwargs=None):
    from concourse.bass_utils import run_bass_kernel_spmd

    nc = _build()
    in_maps, M, fmean, umean = _prepare(inputs)
    res = run_bass_kernel_spmd(nc, in_maps, core_ids=list(range(NCORES)),
                               trace=trace, **(trace_kwargs or {}))
    return _finish(res.results, M, fmean, umean), res


def kernel(**inputs):
    (out, new_M), _ = _run(inputs)
    return out, new_M


# revision 22
# speedup vs baseline: 1.0514x; 1.0118x over previous
"""Trainium2 Bass kernel for nn_ContinuumMemoryCell (scatter_memory).

Data-parallel over batch B across 8 NeuronCores. Device does the three
B-sized matmuls (error = x @ (V_w.T - M), y_pred = x @ M, and the Hebbian
partial dp_i = error_i.T @ x_i) plus the fused elementwise output
out = y_pred + mix * error. Everything O(D*H) or smaller (gate vectors,
sigmoid means, the final new_M AXPY, shard stitching) happens on host.
"""

import sys

if "/opt/trn_rl_repo" not in sys.path:
    sys.path.insert(0, "/opt/trn_rl_repo")

import numpy as np
import ml_dtypes

B, D, H = 16384, 1024, 1024
NCORES = 8
BL = B // NCORES          # 2048 batch rows per core
P = 128                   # partitions
NB = BL // P              # 16 b-tiles per core
NK = D // P               # 8 k-tiles (contraction over d)
NH = H // P               # 8 h-tiles (dp output rows)
FD = 512                  # matmul moving free-dim (one PSUM bank of f32)

_CACHE = {}


def _build():
    """Build + compile the SPMD Bass program (once per process)."""
    if "nc" in _CACHE:
        return _CACHE["nc"]

    import concourse.bacc as bacc
    import concourse.mybir as mybir
    import concourse.tile as tile

    bf16 = mybir.dt.bfloat16
    f32 = mybir.dt.float32

    nc = bacc.Bacc("TRN2", target_bir_lowering=False, debug=False,
                   num_devices=NCORES)

    xT_d = nc.dram_tensor("xT", [D, BL], bf16, kind="ExternalInput")
    xn_d = nc.dram_tensor("xn", [BL, D], bf16, kind="ExternalInput")
    we_d = nc.dram_tensor("we", [D, H], bf16, kind="ExternalInput")
    mm_d = nc.dram_tensor("mm", [D, D], bf16, kind="ExternalInput")
    mx_d = nc.dram_tensor("mx", [P, NB], f32, kind="ExternalInput")
    out_d = nc.dram_tensor("out", [BL, H], f32, kind="ExternalOutput")
    dp_d = nc.dram_tensor("dp", [H, D], f32, kind="ExternalOutput")

    with tile.TileContext(nc) as tc:
        with (
            tc.tile_pool(name="big", bufs=1) as big,
            tc.tile_pool(name="work", bufs=4) as work,
            tc.tile_pool(name="ps", bufs=4, space="PSUM") as ps,
        ):
            # Resident SBUF tensors, one tile per 128-row chunk so Tile
            # tracks DMA->matmul deps at chunk granularity.
            # Resident SBUF tensors. DMA chunks are k-GROUPED with
            # geometric sizes: small leading chunks so the err pass can
            # start early, big trailing chunks to amortize the ~2us
            # per-transfer completion cost (knee ~860KB).
            KG = [[0], [1], [2, 3], [4, 5, 6, 7]]
            MG = [[0, 1, 2, 3], [4, 5, 6, 7]]
            XG = [[0, 1, 2, 3], [4, 5, 6, 7], [8, 9, 10, 11], [12, 13, 14, 15]]

            xT_g = [big.tile([P, len(g) * BL], bf16, tag=f"xTg{j}",
                             name=f"xTg{j}") for j, g in enumerate(KG)]
            we_g = [big.tile([P, len(g) * H], bf16, tag=f"weg{j}",
                             name=f"weg{j}") for j, g in enumerate(KG)]
            mm_g = [big.tile([P, len(g) * D], bf16, tag=f"mmg{j}",
                             name=f"mmg{j}") for j, g in enumerate(MG)]
            xn_g = [big.tile([P, len(g) * D], bf16, tag=f"xng{j}",
                             name=f"xng{j}") for j, g in enumerate(XG)]
            err_sb = [big.tile([P, H], bf16, tag=f"err{i}", name=f"err{i}")
                      for i in range(NB)]
            mx_sb = big.tile([P, NB], f32, tag="mx", name="mx")

            # per-k view: (tile, column offset of that k-slice)
            xT_v, we_v, mm_v, xn_v = {}, {}, {}, {}
            for j, g in enumerate(KG):
                for jj, k in enumerate(g):
                    xT_v[k] = (xT_g[j], jj * BL)
                    we_v[k] = (we_g[j], jj * H)
            for j, g in enumerate(MG):
                for jj, k in enumerate(g):
                    mm_v[k] = (mm_g[j], jj * D)
            for j, g in enumerate(XG):
                for jj, i in enumerate(g):
                    xn_v[i] = (xn_g[j], jj * D)

            def _rows(dram, g):
                a = dram[g[0] * P:(g[0] + len(g)) * P, :]
                return a.rearrange("(g p) b -> p g b", p=P)

            def _gview(tile_, g):
                return tile_[:].rearrange("p (g b) -> p g b", g=len(g))

            # Input loads in need order, round-robined over the three rings;
            # gpsimd's ring additionally carries the out/dp writes later.
            load_eng = [nc.sync, nc.gpsimd, nc.scalar]
            chunks = []
            for j, g in enumerate(KG):
                chunks.append((_gview(xT_g[j], g), _rows(xT_d, g)))
                chunks.append((_gview(we_g[j], g), _rows(we_d, g)))
            for j, (dst, src_) in enumerate(chunks):
                load_eng[j % 3].dma_start(dst, src_)
            nc.gpsimd.dma_start(mx_sb[:], mx_d[:])
            xn_eng = [nc.sync, nc.scalar]
            for j, g in enumerate(MG):
                xn_eng[j % 2].dma_start(_gview(mm_g[j], g), _rows(mm_d, g))
            for j, g in enumerate(XG):
                xn_eng[j % 2].dma_start(_gview(xn_g[j], g), _rows(xn_d, g))

            # Phase 1a: err = x @ (V_w.T - M), stored bf16 in SBUF.
            for i in range(NB):
                pe = ps.tile([P, H], f32, tag="acc", name=f"pe{i}")
                for k in range(NK):
                    xt, xo = xT_v[k]
                    wt, wo = we_v[k]
                    lhs = xt[:, xo + i * P:xo + (i + 1) * P]
                    st, sp = (k == 0), (k == NK - 1)
                    for h2 in range(2):
                        nc.tensor.matmul(
                            pe[:, h2 * FD:(h2 + 1) * FD], lhs,
                            wt[:, wo + h2 * FD:wo + (h2 + 1) * FD],
                            start=st, stop=sp)
                nc.any.tensor_copy(err_sb[i][:], pe[:])

            # Phase 1b: y = x @ M; fused epilogue out = err*mix + y.
            for i in range(NB):
                py = ps.tile([P, D], f32, tag="acc", name=f"py{i}")
                for k in range(NK):
                    xt, xo = xT_v[k]
                    mt, mo = mm_v[k]
                    lhs = xt[:, xo + i * P:xo + (i + 1) * P]
                    st, sp = (k == 0), (k == NK - 1)
                    for h2 in range(2):
                        nc.tensor.matmul(
                            py[:, h2 * FD:(h2 + 1) * FD], lhs,
                            mt[:, mo + h2 * FD:mo + (h2 + 1) * FD],
                            start=st, stop=sp)
                o = work.tile([P, D], f32, tag="o", name=f"o{i}", bufs=6)
                for h2 in range(2):
                    nc.vector.scalar_tensor_tensor(
                        o[:, h2 * FD:(h2 + 1) * FD],
                        err_sb[i][:, h2 * FD:(h2 + 1) * FD],
                        mx_sb[:, i:i + 1],
                        py[:, h2 * FD:(h2 + 1) * FD],
                        mybir.AluOpType.mult, mybir.AluOpType.add)
                nc.gpsimd.dma_start(out_d[i * P:(i + 1) * P, :], o[:])

            # Phase 2: dp[h-tile] = sum_b err[b, h-tile].T @ x[b, :]
            for t in range(NH):
                pd = ps.tile([P, D], f32, tag="acc", name=f"pd{t}")
                for i in range(NB):
                    xnt, xno = xn_v[i]
                    lhs = err_sb[i][:, t * P:(t + 1) * P]
                    st, sp = (i == 0), (i == NB - 1)
                    for h2 in range(2):
                        nc.tensor.matmul(
                            pd[:, h2 * FD:(h2 + 1) * FD], lhs,
                            xnt[:, xno + h2 * FD:xno + (h2 + 1) * FD],
                            start=st, stop=sp)
                dpt = work.tile([P, D], f32, tag="dpt", name=f"dpt{t}")
                for h2 in range(2):
                    nc.any.tensor_copy(dpt[:, h2 * FD:(h2 + 1) * FD],
                                       pd[:, h2 * FD:(h2 + 1) * FD])
                    nc.gpsimd.dma_start(
                        dp_d[t * P:(t + 1) * P, h2 * FD:(h2 + 1) * FD],
                        dpt[:, h2 * FD:(h2 + 1) * FD])

    nc.compile()
    _CACHE["nc"] = nc
    return nc


def _prepare(inputs):
    """Host-side preprocessing: shard + dtype-convert + gate math."""
    x = np.asarray(inputs["x"], np.float32)
    V_w = np.asarray(inputs["V_w"], np.float32)
    M = np.asarray(inputs["M"], np.float32)
    fg_w = np.asarray(inputs["fg_w"], np.float32)
    fg_b = np.asarray(inputs["fg_b"], np.float32)
    ug_w = np.asarray(inputs["ug_w"], np.float32)
    ug_b = np.asarray(inputs["ug_b"], np.float32)
    sm_w = np.asarray(inputs["sm_w"], np.float32)
    sm_b = np.asarray(inputs["sm_b"], np.float32)

    bf16 = ml_dtypes.bfloat16
    VT = V_w.T                                # [D, H]
    we = np.ascontiguousarray(VT - M).astype(bf16)
    mm = np.ascontiguousarray(M).astype(bf16)

    # v @ a == x @ (V_w.T @ a): collapse each gate to one D-vector on x.
    c_f = VT @ fg_w[0, :H] + fg_w[0, H:]
    c_u = VT @ ug_w[0, :H] + ug_w[0, H:]
    c_m = VT @ sm_w[0]
    logits = x @ np.stack([c_f, c_u, c_m], axis=1)       # [B, 3]
    sig = 1.0 / (1.0 + np.exp(-(logits + np.array([fg_b[0], ug_b[0], sm_b[0]]))))
    fmean = float(sig[:, 0].mean())
    umean = float(sig[:, 1].mean())
    mix = sig[:, 2].astype(np.float32)                   # [B]

    xb = x.astype(bf16)
    in_maps = []
    for i in range(NCORES):
        s = slice(i * BL, (i + 1) * BL)
        in_maps.append({
            "xT": np.ascontiguousarray(xb[s].T),
            "xn": np.ascontiguousarray(xb[s]),
            "we": we,
            "mm": mm,
            "mx": np.ascontiguousarray(mix[s].reshape(NB, P).T),
        })
    return in_maps, M, fmean, umean


def _finish(results, M, fmean, umean):
    out = np.concatenate([results[i]["out"] for i in range(NCORES)], axis=0)
    delta = results[0]["dp"].astype(np.float64)
    for i in range(1, NCORES):
        delta += results[i]["dp"]
    delta_mean = (delta / B).astype(np.float32)
    new_M = fmean * M + umean * 0.1 * delta_mean
    return out.astype(np.float32), new_M.astype(np.float32)


def _run(inputs, trace=False, trace_kwargs=None):
    from concourse.bass_utils import run_bass_kernel_spmd

    nc = _build()
    in_maps, M, fmean, umean = _prepare(inputs)
    res = run_bass_kernel_spmd(nc, in_maps, core_ids=list(range(NCORES)),
                               trace=trace, **(trace_kwargs or {}))
    return _finish(res.results, M, fmean, umean), res


def kernel(**inputs):
    (out, new_M), _ = _run(inputs)
    return out, new_M


# revision 31
# speedup vs baseline: 1.2031x; 1.1443x over previous
"""Trainium2 Bass kernel for nn_ContinuumMemoryCell (scatter_memory).

Data-parallel over batch B across 8 NeuronCores. Device does the three
B-sized matmuls (error = x @ (V_w.T - M), y_pred = x @ M, and the Hebbian
partial dp_i = error_i.T @ x_i) plus the fused elementwise output
out = y_pred + mix * error. Everything O(D*H) or smaller (gate vectors,
sigmoid means, the final new_M AXPY, shard stitching) happens on host.
"""

import sys

if "/opt/trn_rl_repo" not in sys.path:
    sys.path.insert(0, "/opt/trn_rl_repo")

import numpy as np
import ml_dtypes

B, D, H = 16384, 1024, 1024
NCORES = 8
BL = B // NCORES          # 2048 batch rows per core
P = 128                   # partitions
NB = BL // P              # 16 b-tiles per core
NK = D // P               # 8 k-tiles (contraction over d)
NH = H // P               # 8 h-tiles (dp output rows)
FD = 512                  # matmul moving free-dim (one PSUM bank of f32)
OUT_SCALE = 4096.0        # 2^12: lifts M into fp8e4m3 normal range

_CACHE = {}


def _build():
    """Build + compile the SPMD Bass program (once per process)."""
    if "nc" in _CACHE:
        return _CACHE["nc"]

    import concourse.bacc as bacc
    import concourse.mybir as mybir
    import concourse.tile as tile

    bf16 = mybir.dt.bfloat16
    f32 = mybir.dt.float32

    nc = bacc.Bacc("TRN2", target_bir_lowering=False, debug=False,
                   num_devices=NCORES)

    xT_d = nc.dram_tensor("xT", [D, BL], bf16, kind="ExternalInput")
    xn_d = nc.dram_tensor("xn", [BL, D], bf16, kind="ExternalInput")
    we_d = nc.dram_tensor("we", [D, H], bf16, kind="ExternalInput")
    fp8 = mybir.dt.float8e4
    # x and M*2^12 in fp8 e4m3, pre-paired along d for DoubleRow matmuls:
    # row k' holds d = 2k' and 2k'+1.
    x8_d = nc.dram_tensor("x8", [D // 2, 2 * BL], fp8, kind="ExternalInput")
    m8_d = nc.dram_tensor("m8", [D // 2, 2 * D], fp8, kind="ExternalInput")
    mx_d = nc.dram_tensor("mx", [P, NB], f32, kind="ExternalInput")
    out_d = nc.dram_tensor("out", [BL, H], f32, kind="ExternalOutput")
    dp_d = nc.dram_tensor("dp", [H, D], f32, kind="ExternalOutput")

    with tile.TileContext(nc) as tc:
        with (
            tc.tile_pool(name="big", bufs=1) as big,
            tc.tile_pool(name="work", bufs=4) as work,
            tc.tile_pool(name="ps", bufs=4, space="PSUM") as ps,
        ):
            # Resident SBUF tensors, one tile per 128-row chunk so Tile
            # tracks DMA->matmul deps at chunk granularity.
            # Resident SBUF tensors. DMA chunks are k-GROUPED with
            # geometric sizes: small leading chunks so the err pass can
            # start early, big trailing chunks to amortize the ~2us
            # per-transfer completion cost (knee ~860KB).
            KG = [[0], [1], [2, 3], [4, 5, 6, 7]]
            MG = [[0, 1, 2, 3], [4, 5, 6, 7]]
            XG = [[0, 1, 2, 3], [4, 5, 6, 7], [8, 9, 10, 11], [12, 13, 14, 15]]

            xT_g = [big.tile([P, len(g) * BL], bf16, tag=f"xTg{j}",
                             name=f"xTg{j}") for j, g in enumerate(KG)]
            we_g = [big.tile([P, len(g) * H], bf16, tag=f"weg{j}",
                             name=f"weg{j}") for j, g in enumerate(KG)]
            x8_g = [big.tile([P, 2 * BL], fp8, tag=f"x8g{j}",
                             name=f"x8g{j}") for j in range(4)]
            m8_g = [big.tile([P, 2 * D], fp8, tag=f"m8g{j}",
                             name=f"m8g{j}") for j in range(4)]
            xn_g = [big.tile([P, len(g) * D], bf16, tag=f"xng{j}",
                             name=f"xng{j}") for j, g in enumerate(XG)]
            err_sb = [big.tile([P, H], bf16, tag=f"err{i}", name=f"err{i}")
                      for i in range(NB)]
            mx_sb = big.tile([P, NB], f32, tag="mx", name="mx")

            # per-k view: (tile, column offset of that k-slice)
            xT_v, we_v, xn_v = {}, {}, {}
            for j, g in enumerate(KG):
                for jj, k in enumerate(g):
                    xT_v[k] = (xT_g[j], jj * BL)
                    we_v[k] = (we_g[j], jj * H)
            for j, g in enumerate(XG):
                for jj, i in enumerate(g):
                    xn_v[i] = (xn_g[j], jj * D)

            def _rows(dram, g):
                a = dram[g[0] * P:(g[0] + len(g)) * P, :]
                return a.rearrange("(g p) b -> p g b", p=P)

            def _gview(tile_, g):
                return tile_[:].rearrange("p (g b) -> p g b", g=len(g))

            # Input loads in need order, round-robined over the three rings;
            # gpsimd's ring additionally carries the out/dp writes later.
            load_eng = [nc.sync, nc.gpsimd, nc.scalar]
            chunks = []
            for j, g in enumerate(KG):
                chunks.append((_gview(xT_g[j], g), _rows(xT_d, g)))
                chunks.append((_gview(we_g[j], g), _rows(we_d, g)))
            for j, (dst, src_) in enumerate(chunks):
                load_eng[j % 3].dma_start(dst, src_)
            nc.gpsimd.dma_start(mx_sb[:], mx_d[:])
            xn_eng = [nc.sync, nc.scalar]
            for j in range(4):
                xn_eng[j % 2].dma_start(x8_g[j][:], x8_d[j * P:(j + 1) * P, :])
                xn_eng[(j + 1) % 2].dma_start(m8_g[j][:],
                                              m8_d[j * P:(j + 1) * P, :])
            for j, g in enumerate(XG):
                xn_eng[j % 2].dma_start(_gview(xn_g[j], g), _rows(xn_d, g))

            # Phase 1a: err = x @ (V_w.T - M), stored bf16 in SBUF.
            for i in range(NB):
                pe = ps.tile([P, H], f32, tag="acc", name=f"pe{i}")
                for k in range(NK):
                    xt, xo = xT_v[k]
                    wt, wo = we_v[k]
                    lhs = xt[:, xo + i * P:xo + (i + 1) * P]
                    st, sp = (k == 0), (k == NK - 1)
                    for h2 in range(2):
                        nc.tensor.matmul(
                            pe[:, h2 * FD:(h2 + 1) * FD], lhs,
                            wt[:, wo + h2 * FD:wo + (h2 + 1) * FD],
                            start=st, stop=sp)
                nc.any.tensor_copy(err_sb[i][:], pe[:])

            # Phase 1b: y*2^12 = x @ (M*2^12) in fp8 DoubleRow (2 d-rows per
            # PE cell, half the instructions); epilogue out*2^12 =
            # err*(mix*2^12) + y*2^12, rescaled on host.
            for i in range(NB):
                py = ps.tile([P, D], f32, tag="acc", name=f"py{i}")
                for kg in range(4):
                    lhs3 = x8_g[kg][:].rearrange("p (two b) -> p two b",
                                                 two=2)[:, :, i * P:(i + 1) * P]
                    m3 = m8_g[kg][:].rearrange("p (two n) -> p two n", two=2)
                    st, sp = (kg == 0), (kg == 3)
                    for h2 in range(2):
                        nc.tensor.matmul(
                            py[:, h2 * FD:(h2 + 1) * FD], lhs3,
                            m3[:, :, h2 * FD:(h2 + 1) * FD],
                            start=st, stop=sp,
                            perf_mode=mybir.MatmulPerfMode.DoubleRow)
                o = work.tile([P, D], f32, tag="o", name=f"o{i}", bufs=6)
                for h2 in range(2):
                    nc.vector.scalar_tensor_tensor(
                        o[:, h2 * FD:(h2 + 1) * FD],
                        err_sb[i][:, h2 * FD:(h2 + 1) * FD],
                        mx_sb[:, i:i + 1],
                        py[:, h2 * FD:(h2 + 1) * FD],
                        mybir.AluOpType.mult, mybir.AluOpType.add)
                nc.gpsimd.dma_start(out_d[i * P:(i + 1) * P, :], o[:])

            # Phase 2: dp[h-tile] = sum_b err[b, h-tile].T @ x[b, :]
            for t in range(NH):
                pd = ps.tile([P, D], f32, tag="acc", name=f"pd{t}")
                for i in range(NB):
                    xnt, xno = xn_v[i]
                    lhs = err_sb[i][:, t * P:(t + 1) * P]
                    st, sp = (i == 0), (i == NB - 1)
                    for h2 in range(2):
                        nc.tensor.matmul(
                            pd[:, h2 * FD:(h2 + 1) * FD], lhs,
                            xnt[:, xno + h2 * FD:xno + (h2 + 1) * FD],
                            start=st, stop=sp)
                dpt = work.tile([P, D], f32, tag="dpt", name=f"dpt{t}")
                for h2 in range(2):
                    nc.any.tensor_copy(dpt[:, h2 * FD:(h2 + 1) * FD],
                                       pd[:, h2 * FD:(h2 + 1) * FD])
                    nc.gpsimd.dma_start(
                        dp_d[t * P:(t + 1) * P, h2 * FD:(h2 + 1) * FD],
                        dpt[:, h2 * FD:(h2 + 1) * FD])

    nc.compile()
    _CACHE["nc"] = nc
    return nc


def _prepare(inputs):
    """Host-side preprocessing: shard + dtype-convert + gate math."""
    x = np.asarray(inputs["x"], np.float32)
    V_w = np.asarray(inputs["V_w"], np.float32)
    M = np.asarray(inputs["M"], np.float32)
    fg_w = np.asarray(inputs["fg_w"], np.float32)
    fg_b = np.asarray(inputs["fg_b"], np.float32)
    ug_w = np.asarray(inputs["ug_w"], np.float32)
    ug_b = np.asarray(inputs["ug_b"], np.float32)
    sm_w = np.asarray(inputs["sm_w"], np.float32)
    sm_b = np.asarray(inputs["sm_b"], np.float32)

    bf16 = ml_dtypes.bfloat16
    fp8 = ml_dtypes.float8_e4m3
    VT = V_w.T                                # [D, H]
    we = np.ascontiguousarray(VT - M).astype(bf16)
    m8 = np.clip(M * OUT_SCALE, -240, 240).astype(fp8).reshape(D // 2, 2 * D)

    # v @ a == x @ (V_w.T @ a): collapse each gate to one D-vector on x.
    c_f = VT @ fg_w[0, :H] + fg_w[0, H:]
    c_u = VT @ ug_w[0, :H] + ug_w[0, H:]
    c_m = VT @ sm_w[0]
    logits = x @ np.stack([c_f, c_u, c_m], axis=1)       # [B, 3]
    sig = 1.0 / (1.0 + np.exp(-(logits + np.array([fg_b[0], ug_b[0], sm_b[0]]))))
    fmean = float(sig[:, 0].mean())
    umean = float(sig[:, 1].mean())
    mix = sig[:, 2].astype(np.float32)                   # [B]

    xb = x.astype(bf16)
    x8 = np.clip(x, -240, 240).astype(fp8)
    mixs = (mix * OUT_SCALE).astype(np.float32)
    in_maps = []
    for i in range(NCORES):
        s = slice(i * BL, (i + 1) * BL)
        in_maps.append({
            "xT": np.ascontiguousarray(xb[s].T),
            "xn": np.ascontiguousarray(xb[s]),
            "we": we,
            "x8": np.ascontiguousarray(x8[s].T).reshape(D // 2, 2 * BL),
            "m8": m8,
            "mx": np.ascontiguousarray(mixs[s].reshape(NB, P).T),
        })
    return in_maps, M, fmean, umean


def _finish(results, M, fmean, umean):
    out = np.concatenate([results[i]["out"] for i in range(NCORES)], axis=0)
    out *= 1.0 / OUT_SCALE
    delta = results[0]["dp"].astype(np.float64)
    for i in range(1, NCORES):
        delta += results[i]["dp"]
    delta_mean = (delta / B).astype(np.float32)
    new_M = fmean * M + umean * 0.1 * delta_mean
    return out.astype(np.float32), new_M.astype(np.float32)


def _run(inputs, trace=False, trace_kwargs=None):
    from concourse.bass_utils import run_bass_kernel_spmd

    nc = _build()
    in_maps, M, fmean, umean = _prepare(inputs)
    res = run_bass_kernel_spmd(nc, in_maps, core_ids=list(range(NCORES)),
                               trace=trace, **(trace_kwargs or {}))
    return _finish(res.results, M, fmean, umean), res


def kernel(**inputs):
    (out, new_M), _ = _run(inputs)
    return out, new_M


# revision 34
# speedup vs baseline: 1.2072x; 1.0034x over previous
"""Trainium2 Bass kernel for nn_ContinuumMemoryCell (scatter_memory).

Data-parallel over batch B across 8 NeuronCores. Device does the three
B-sized matmuls (error = x @ (V_w.T - M), y_pred = x @ M, and the Hebbian
partial dp_i = error_i.T @ x_i) plus the fused elementwise output
out = y_pred + mix * error. Everything O(D*H) or smaller (gate vectors,
sigmoid means, the final new_M AXPY, shard stitching) happens on host.
"""

import sys

if "/opt/trn_rl_repo" not in sys.path:
    sys.path.insert(0, "/opt/trn_rl_repo")

import numpy as np
import ml_dtypes

B, D, H = 16384, 1024, 1024
NCORES = 8
BL = B // NCORES          # 2048 batch rows per core
P = 128                   # partitions
NB = BL // P              # 16 b-tiles per core
NK = D // P               # 8 k-tiles (contraction over d)
NH = H // P               # 8 h-tiles (dp output rows)
FD = 512                  # matmul moving free-dim (one PSUM bank of f32)
OUT_SCALE = 4096.0        # 2^12: lifts M into fp8e4m3 normal range

_CACHE = {}


def _build():
    """Build + compile the SPMD Bass program (once per process)."""
    if "nc" in _CACHE:
        return _CACHE["nc"]

    import concourse.bacc as bacc
    import concourse.mybir as mybir
    import concourse.tile as tile

    bf16 = mybir.dt.bfloat16
    f32 = mybir.dt.float32

    nc = bacc.Bacc("TRN2", target_bir_lowering=False, debug=False,
                   num_devices=NCORES)

    xT_d = nc.dram_tensor("xT", [D, BL], bf16, kind="ExternalInput")
    xn_d = nc.dram_tensor("xn", [BL, D], bf16, kind="ExternalInput")
    we_d = nc.dram_tensor("we", [D, H], bf16, kind="ExternalInput")
    fp8 = mybir.dt.float8e4
    # x and M*2^12 in fp8 e4m3, pre-paired along d for DoubleRow matmuls:
    # row k' holds d = 2k' and 2k'+1.
    x8_d = nc.dram_tensor("x8", [D // 2, 2 * BL], fp8, kind="ExternalInput")
    m8_d = nc.dram_tensor("m8", [D // 2, 2 * D], fp8, kind="ExternalInput")
    mx_d = nc.dram_tensor("mx", [P, NB], f32, kind="ExternalInput")
    out_d = nc.dram_tensor("out", [BL, H], f32, kind="ExternalOutput")
    dp_d = nc.dram_tensor("dp", [H, D], f32, kind="ExternalOutput")

    with tile.TileContext(nc) as tc:
        with (
            tc.tile_pool(name="big", bufs=1) as big,
            tc.tile_pool(name="work", bufs=4) as work,
            tc.tile_pool(name="ps", bufs=4, space="PSUM") as ps,
        ):
            # Resident SBUF tensors, one tile per 128-row chunk so Tile
            # tracks DMA->matmul deps at chunk granularity.
            # Resident SBUF tensors. DMA chunks are k-GROUPED with
            # geometric sizes: small leading chunks so the err pass can
            # start early, big trailing chunks to amortize the ~2us
            # per-transfer completion cost (knee ~860KB).
            KG = [[0], [1], [2, 3], [4, 5, 6, 7]]
            MG = [[0, 1, 2, 3], [4, 5, 6, 7]]
            XG = [[0, 1, 2, 3], [4, 5, 6, 7], [8, 9, 10, 11], [12, 13, 14, 15]]

            xT_g = [big.tile([P, len(g) * BL], bf16, tag=f"xTg{j}",
                             name=f"xTg{j}") for j, g in enumerate(KG)]
            we_g = [big.tile([P, len(g) * H], bf16, tag=f"weg{j}",
                             name=f"weg{j}") for j, g in enumerate(KG)]
            x8_g = [big.tile([P, 2 * BL], fp8, tag=f"x8g{j}",
                             name=f"x8g{j}") for j in range(4)]
            m8_g = [big.tile([P, 2 * D], fp8, tag=f"m8g{j}",
                             name=f"m8g{j}") for j in range(4)]
            xn_g = [big.tile([P, len(g) * D], bf16, tag=f"xng{j}",
                             name=f"xng{j}") for j, g in enumerate(XG)]
            err_sb = [big.tile([P, H], bf16, tag=f"err{i}", name=f"err{i}")
                      for i in range(NB)]
            mx_sb = big.tile([P, NB], f32, tag="mx", name="mx")

            # per-k view: (tile, column offset of that k-slice)
            xT_v, we_v, xn_v = {}, {}, {}
            for j, g in enumerate(KG):
                for jj, k in enumerate(g):
                    xT_v[k] = (xT_g[j], jj * BL)
                    we_v[k] = (we_g[j], jj * H)
            for j, g in enumerate(XG):
                for jj, i in enumerate(g):
                    xn_v[i] = (xn_g[j], jj * D)

            def _rows(dram, g):
                a = dram[g[0] * P:(g[0] + len(g)) * P, :]
                return a.rearrange("(g p) b -> p g b", p=P)

            def _gview(tile_, g):
                return tile_[:].rearrange("p (g b) -> p g b", g=len(g))

            # Input loads: hand-balanced over the three rings so chunk k of
            # xT and we land together and in ascending k. gpsimd's ring
            # additionally carries the out writes later.
            nc.sync.dma_start(_gview(xT_g[0], KG[0]), _rows(xT_d, KG[0]))
            nc.sync.dma_start(_gview(we_g[1], KG[1]), _rows(we_d, KG[1]))
            nc.sync.dma_start(_gview(xT_g[3], KG[3]), _rows(xT_d, KG[3]))
            nc.gpsimd.dma_start(_gview(we_g[0], KG[0]), _rows(we_d, KG[0]))
            nc.gpsimd.dma_start(_gview(we_g[2], KG[2]), _rows(we_d, KG[2]))
            nc.gpsimd.dma_start(_gview(we_g[3], KG[3]), _rows(we_d, KG[3]))
            nc.scalar.dma_start(_gview(xT_g[1], KG[1]), _rows(xT_d, KG[1]))
            nc.scalar.dma_start(_gview(xT_g[2], KG[2]), _rows(xT_d, KG[2]))
            nc.gpsimd.dma_start(mx_sb[:], mx_d[:])
            xn_eng = [nc.sync, nc.scalar]
            for j in range(4):
                xn_eng[j % 2].dma_start(x8_g[j][:], x8_d[j * P:(j + 1) * P, :])
                xn_eng[(j + 1) % 2].dma_start(m8_g[j][:],
                                              m8_d[j * P:(j + 1) * P, :])
            for j, g in enumerate(XG):
                xn_eng[j % 2].dma_start(_gview(xn_g[j], g), _rows(xn_d, g))

            # Phase 1a: err = x @ (V_w.T - M), stored bf16 in SBUF.
            for i in range(NB):
                pe = ps.tile([P, H], f32, tag="acc", name=f"pe{i}")
                for k in range(NK):
                    xt, xo = xT_v[k]
                    wt, wo = we_v[k]
                    lhs = xt[:, xo + i * P:xo + (i + 1) * P]
                    st, sp = (k == 0), (k == NK - 1)
                    for h2 in range(2):
                        nc.tensor.matmul(
                            pe[:, h2 * FD:(h2 + 1) * FD], lhs,
                            wt[:, wo + h2 * FD:wo + (h2 + 1) * FD],
                            start=st, stop=sp)
                nc.vector.tensor_copy(err_sb[i][:], pe[:])

            # Phase 1b: y*2^12 = x @ (M*2^12) in fp8 DoubleRow (2 d-rows per
            # PE cell, half the instructions); epilogue out*2^12 =
            # err*(mix*2^12) + y*2^12, rescaled on host.
            for i in range(NB):
                py = ps.tile([P, D], f32, tag="acc", name=f"py{i}")
                for kg in range(4):
                    lhs3 = x8_g[kg][:].rearrange("p (two b) -> p two b",
                                                 two=2)[:, :, i * P:(i + 1) * P]
                    m3 = m8_g[kg][:].rearrange("p (two n) -> p two n", two=2)
                    st, sp = (kg == 0), (kg == 3)
                    for h2 in range(2):
                        nc.tensor.matmul(
                            py[:, h2 * FD:(h2 + 1) * FD], lhs3,
                            m3[:, :, h2 * FD:(h2 + 1) * FD],
                            start=st, stop=sp,
                            perf_mode=mybir.MatmulPerfMode.DoubleRow)
                o = work.tile([P, D], f32, tag="o", name=f"o{i}", bufs=6)
                for h2 in range(2):
                    nc.vector.scalar_tensor_tensor(
                        o[:, h2 * FD:(h2 + 1) * FD],
                        err_sb[i][:, h2 * FD:(h2 + 1) * FD],
                        mx_sb[:, i:i + 1],
                        py[:, h2 * FD:(h2 + 1) * FD],
                        mybir.AluOpType.mult, mybir.AluOpType.add)
                nc.gpsimd.dma_start(out_d[i * P:(i + 1) * P, :], o[:])

            # Phase 2: dp[h-tile] = sum_b err[b, h-tile].T @ x[b, :]
            for t in range(NH):
                pd = ps.tile([P, D], f32, tag="acc", name=f"pd{t}")
                for i in range(NB):
                    xnt, xno = xn_v[i]
                    lhs = err_sb[i][:, t * P:(t + 1) * P]
                    st, sp = (i == 0), (i == NB - 1)
                    for h2 in range(2):
                        nc.tensor.matmul(
                            pd[:, h2 * FD:(h2 + 1) * FD], lhs,
                            xnt[:, xno + h2 * FD:xno + (h2 + 1) * FD],
                            start=st, stop=sp)
                dpt = work.tile([P, D], f32, tag="dpt", name=f"dpt{t}")
                dp_eng = [nc.sync, nc.scalar]
                if t < NH - 1:
                    for h2 in range(2):
                        nc.vector.tensor_copy(dpt[:, h2 * FD:(h2 + 1) * FD],
                                              pd[:, h2 * FD:(h2 + 1) * FD])
                        dp_eng[h2].dma_start(
                            dp_d[t * P:(t + 1) * P, h2 * FD:(h2 + 1) * FD],
                            dpt[:, h2 * FD:(h2 + 1) * FD])
                else:
                    # Last h-tile gates the kernel tail: drain in quarters
                    # across both idle HWDGE rings to shorten the chain.
                    Q = FD // 2
                    for q in range(4):
                        nc.vector.tensor_copy(dpt[:, q * Q:(q + 1) * Q],
                                              pd[:, q * Q:(q + 1) * Q])
                        dp_eng[q % 2].dma_start(
                            dp_d[t * P:(t + 1) * P, q * Q:(q + 1) * Q],
                            dpt[:, q * Q:(q + 1) * Q])

    nc.compile()
    _CACHE["nc"] = nc
    return nc


def _prepare(inputs):
    """Host-side preprocessing: shard + dtype-convert + gate math."""
    x = np.asarray(inputs["x"], np.float32)
    V_w = np.asarray(inputs["V_w"], np.float32)
    M = np.asarray(inputs["M"], np.float32)
    fg_w = np.asarray(inputs["fg_w"], np.float32)
    fg_b = np.asarray(inputs["fg_b"], np.float32)
    ug_w = np.asarray(inputs["ug_w"], np.float32)
    ug_b = np.asarray(inputs["ug_b"], np.float32)
    sm_w = np.asarray(inputs["sm_w"], np.float32)
    sm_b = np.asarray(inputs["sm_b"], np.float32)

    bf16 = ml_dtypes.bfloat16
    fp8 = ml_dtypes.float8_e4m3
    VT = V_w.T                                # [D, H]
    we = np.ascontiguousarray(VT - M).astype(bf16)
    m8 = np.clip(M * OUT_SCALE, -240, 240).astype(fp8).reshape(D // 2, 2 * D)

    # v @ a == x @ (V_w.T @ a): collapse each gate to one D-vector on x.
    c_f = VT @ fg_w[0, :H] + fg_w[0, H:]
    c_u = VT @ ug_w[0, :H] + ug_w[0, H:]
    c_m = VT @ sm_w[0]
    logits = x @ np.stack([c_f, c_u, c_m], axis=1)       # [B, 3]
    sig = 1.0 / (1.0 + np.exp(-(logits + np.array([fg_b[0], ug_b[0], sm_b[0]]))))
    fmean = float(sig[:, 0].mean())
    umean = float(sig[:, 1].mean())
    mix = sig[:, 2].astype(np.float32)                   # [B]

    xb = x.astype(bf16)
    x8 = np.clip(x, -240, 240).astype(fp8)
    mixs = (mix * OUT_SCALE).astype(np.float32)
    in_maps = []
    for i in range(NCORES):
        s = slice(i * BL, (i + 1) * BL)
        in_maps.append({
            "xT": np.ascontiguousarray(xb[s].T),
            "xn": np.ascontiguousarray(xb[s]),
            "we": we,
            "x8": np.ascontiguousarray(x8[s].T).reshape(D // 2, 2 * BL),
            "m8": m8,
            "mx": np.ascontiguousarray(mixs[s].reshape(NB, P).T),
        })
    return in_maps, M, fmean, umean


def _finish(results, M, fmean, umean):
    out = np.concatenate([results[i]["out"] for i in range(NCORES)], axis=0)
    out *= 1.0 / OUT_SCALE
    delta = results[0]["dp"].astype(np.float64)
    for i in range(1, NCORES):
        delta += results[i]["dp"]
    delta_mean = (delta / B).astype(np.float32)
    new_M = fmean * M + umean * 0.1 * delta_mean
    return out.astype(np.float32), new_M.astype(np.float32)


def _run(inputs, trace=False, trace_kwargs=None):
    from concourse.bass_utils import run_bass_kernel_spmd

    nc = _build()
    in_maps, M, fmean, umean = _prepare(inputs)
    res = run_bass_kernel_spmd(nc, in_maps, core_ids=list(range(NCORES)),
                               trace=trace, **(trace_kwargs or {}))
    return _finish(res.results, M, fmean, umean), res


def kernel(**inputs):
    (out, new_M), _ = _run(inputs)
    return out, new_M


# revision 35
# speedup vs baseline: 1.2853x; 1.0647x over previous
"""Trainium2 Bass kernel for nn_ContinuumMemoryCell (scatter_memory).

Data-parallel over batch B across 8 NeuronCores. Device does the three
B-sized matmuls (error = x @ (V_w.T - M) in bf16, y_pred = x @ M in fp8
DoubleRow, and the Hebbian partial dp_i = error_i.T @ x_i in bf16) plus
the fused elementwise output out = y_pred + mix * error. Everything
O(D*H) or smaller (gate vectors, sigmoid means, the final new_M AXPY,
shard stitching) happens on host.

DMA strategy (measured): one HWDGE ring moves ~425 GB/s with big
transfers and parallel rings don't add bandwidth (HBM-bound), so all
input loads ride the sync ring as a few large chunks in need order;
out writes ride scalar's ring; dp writes ride gpsimd.
"""

import sys

if "/opt/trn_rl_repo" not in sys.path:
    sys.path.insert(0, "/opt/trn_rl_repo")

import numpy as np
import ml_dtypes

B, D, H = 16384, 1024, 1024
NCORES = 8
BL = B // NCORES          # 2048 batch rows per core
P = 128                   # partitions
NB = BL // P              # 16 b-tiles per core
NK = D // P               # 8 k-tiles (contraction over d)
NH = H // P               # 8 h-tiles (dp output rows)
FD = 512                  # matmul moving free-dim (one PSUM bank of f32)
OUT_SCALE = 4096.0        # 2^12: lifts M into fp8e4m3 normal range

# k-tile groups for the pass-A input stream: small leading chunks so the
# err pass starts early, larger trailing chunks to amortize per-transfer
# fixed cost.
KGROUPS = [[0], [1], [2, 3], [4, 5], [6, 7]]

_CACHE = {}


def _build():
    """Build + compile the SPMD Bass program (once per process)."""
    if "nc" in _CACHE:
        return _CACHE["nc"]

    import concourse.bacc as bacc
    import concourse.mybir as mybir
    import concourse.tile as tile

    bf16 = mybir.dt.bfloat16
    f32 = mybir.dt.float32
    fp8 = mybir.dt.float8e4

    nc = bacc.Bacc("TRN2", target_bir_lowering=False, debug=False,
                   num_devices=NCORES)

    # wx: per k-group [xT rows | we rows] pre-interleaved on host so the
    # whole pass-A stream is contiguous column chunks of one tensor.
    WXCOLS = NK * (BL + H)
    wx_d = nc.dram_tensor("wx", [P, WXCOLS], bf16, kind="ExternalInput")
    xn_d = nc.dram_tensor("xn", [BL, D], bf16, kind="ExternalInput")
    # x and M*2^12 in fp8 e4m3, pre-paired along d for DoubleRow matmuls:
    # row k' of the logical [D/2, 2, *] holds d = 2k' and 2k'+1.
    x8_d = nc.dram_tensor("x8", [D // 2, 2 * BL], fp8, kind="ExternalInput")
    m8_d = nc.dram_tensor("m8", [D // 2, 2 * D], fp8, kind="ExternalInput")
    mx_d = nc.dram_tensor("mx", [P, NB], f32, kind="ExternalInput")
    out_d = nc.dram_tensor("out", [BL, H], f32, kind="ExternalOutput")
    dp_d = nc.dram_tensor("dp", [H, D], f32, kind="ExternalOutput")

    with tile.TileContext(nc) as tc:
        with (
            tc.tile_pool(name="big", bufs=1) as big,
            tc.tile_pool(name="work", bufs=4) as work,
            tc.tile_pool(name="ps", bufs=4, space="PSUM") as ps,
        ):
            wx_g = [big.tile([P, len(g) * (BL + H)], bf16, tag=f"wxg{j}",
                             name=f"wxg{j}") for j, g in enumerate(KGROUPS)]
            xn_sb = big.tile([P, NB * D], bf16, tag="xn", name="xn")
            x8_sb = big.tile([P, 4 * 2 * BL], fp8, tag="x8", name="x8")
            m8_sb = big.tile([P, 4 * 2 * D], fp8, tag="m8", name="m8")
            err_sb = [big.tile([P, H], bf16, tag=f"err{i}", name=f"err{i}")
                      for i in range(NB)]
            mx_sb = big.tile([P, NB], f32, tag="mx", name="mx")

            # per-k views into the wx group tiles
            xT_v, we_v = {}, {}
            for j, g in enumerate(KGROUPS):
                n = len(g)
                for jj, k in enumerate(g):
                    xT_v[k] = (wx_g[j], jj * BL)
                    we_v[k] = (wx_g[j], n * BL + jj * H)

            # All input loads on the sync HWDGE ring, need order.
            off = 0
            for j, g in enumerate(KGROUPS):
                ncols = len(g) * (BL + H)
                nc.sync.dma_start(wx_g[j][:], wx_d[:, off:off + ncols])
                off += ncols
            nc.gpsimd.dma_start(mx_sb[:], mx_d[:])
            nc.sync.dma_start(
                x8_sb[:].rearrange("p (g c) -> p g c", g=4),
                x8_d.rearrange("(g p) c -> p g c", p=P))
            nc.sync.dma_start(
                m8_sb[:].rearrange("p (g c) -> p g c", g=4),
                m8_d.rearrange("(g p) c -> p g c", p=P))
            nc.sync.dma_start(
                xn_sb[:].rearrange("p (t d) -> p t d", t=NB),
                xn_d.rearrange("(t p) d -> p t d", p=P))

            # Phase 1a: err = x @ (V_w.T - M), stored bf16 in SBUF.
            for i in range(NB):
                pe = ps.tile([P, H], f32, tag="acc", name=f"pe{i}")
                for k in range(NK):
                    xt, xo = xT_v[k]
                    wt, wo = we_v[k]
                    lhs = xt[:, xo + i * P:xo + (i + 1) * P]
                    st, sp = (k == 0), (k == NK - 1)
                    for h2 in range(2):
                        nc.tensor.matmul(
                            pe[:, h2 * FD:(h2 + 1) * FD], lhs,
                            wt[:, wo + h2 * FD:wo + (h2 + 1) * FD],
                            start=st, stop=sp)
                nc.vector.tensor_copy(err_sb[i][:], pe[:])

            # Phase 1b: y*2^12 = x @ (M*2^12) in fp8 DoubleRow (2 d-rows
            # per PE cell, half the instructions); epilogue out*2^12 =
            # err*(mix*2^12) + y*2^12, rescaled on host.
            x8_4 = x8_sb[:].rearrange("p (g two b) -> p g two b", g=4, two=2)
            m8_4 = m8_sb[:].rearrange("p (g two n) -> p g two n", g=4, two=2)
            for i in range(NB):
                py = ps.tile([P, D], f32, tag="acc", name=f"py{i}")
                for kg in range(4):
                    lhs3 = x8_4[:, kg, :, i * P:(i + 1) * P]
                    st, sp = (kg == 0), (kg == 3)
                    for h2 in range(2):
                        nc.tensor.matmul(
                            py[:, h2 * FD:(h2 + 1) * FD], lhs3,
                            m8_4[:, kg, :, h2 * FD:(h2 + 1) * FD],
                            start=st, stop=sp,
                            perf_mode=mybir.MatmulPerfMode.DoubleRow)
                o = work.tile([P, D], f32, tag="o", name=f"o{i}", bufs=6)
                for h2 in range(2):
                    nc.vector.scalar_tensor_tensor(
                        o[:, h2 * FD:(h2 + 1) * FD],
                        err_sb[i][:, h2 * FD:(h2 + 1) * FD],
                        mx_sb[:, i:i + 1],
                        py[:, h2 * FD:(h2 + 1) * FD],
                        mybir.AluOpType.mult, mybir.AluOpType.add)
                nc.scalar.dma_start(out_d[i * P:(i + 1) * P, :], o[:])

            # Phase 2: dp[h-tile] = sum_b err[b, h-tile].T @ x[b, :]
            for t in range(NH):
                pd = ps.tile([P, D], f32, tag="acc", name=f"pd{t}")
                for i in range(NB):
                    lhs = err_sb[i][:, t * P:(t + 1) * P]
                    st, sp = (i == 0), (i == NB - 1)
                    for h2 in range(2):
                        nc.tensor.matmul(
                            pd[:, h2 * FD:(h2 + 1) * FD], lhs,
                            xn_sb[:, i * D + h2 * FD:i * D + (h2 + 1) * FD],
                            start=st, stop=sp)
                dpt = work.tile([P, D], f32, tag="dpt", name=f"dpt{t}")
                if t < NH - 1:
                    for h2 in range(2):
                        nc.vector.tensor_copy(dpt[:, h2 * FD:(h2 + 1) * FD],
                                              pd[:, h2 * FD:(h2 + 1) * FD])
                        nc.gpsimd.dma_start(
                            dp_d[t * P:(t + 1) * P, h2 * FD:(h2 + 1) * FD],
                            dpt[:, h2 * FD:(h2 + 1) * FD])
                else:
                    # Last h-tile gates the kernel tail: drain in quarters
                    # across both idle HWDGE rings to shorten the chain.
                    Q = FD // 2
                    dp_eng = [nc.sync, nc.scalar]
                    for q in range(4):
                        nc.vector.tensor_copy(dpt[:, q * Q:(q + 1) * Q],
                                              pd[:, q * Q:(q + 1) * Q])
                        dp_eng[q % 2].dma_start(
                            dp_d[t * P:(t + 1) * P, q * Q:(q + 1) * Q],
                            dpt[:, q * Q:(q + 1) * Q])

    nc.compile()
    _CACHE["nc"] = nc
    return nc


def _prepare(inputs):
    """Host-side preprocessing: shard + dtype-convert + gate math."""
    x = np.asarray(inputs["x"], np.float32)
    V_w = np.asarray(inputs["V_w"], np.float32)
    M = np.asarray(inputs["M"], np.float32)
    fg_w = np.asarray(inputs["fg_w"], np.float32)
    fg_b = np.asarray(inputs["fg_b"], np.float32)
    ug_w = np.asarray(inputs["ug_w"], np.float32)
    ug_b = np.asarray(inputs["ug_b"], np.float32)
    sm_w = np.asarray(inputs["sm_w"], np.float32)
    sm_b = np.asarray(inputs["sm_b"], np.float32)

    bf16 = ml_dtypes.bfloat16
    fp8 = ml_dtypes.float8_e4m3
    VT = V_w.T                                # [D, H]
    we = np.ascontiguousarray(VT - M).astype(bf16)
    m8 = np.clip(M * OUT_SCALE, -240, 240).astype(fp8).reshape(D // 2, 2 * D)

    # we parts per k-group, shared across cores: [P, n*H] each
    we_parts = []
    for g in KGROUPS:
        n = len(g)
        wp = we[g[0] * P:(g[0] + n) * P].reshape(n, P, H)
        we_parts.append(wp.transpose(1, 0, 2).reshape(P, n * H))

    # v @ a == x @ (V_w.T @ a): collapse each gate to one D-vector on x.
    c_f = VT @ fg_w[0, :H] + fg_w[0, H:]
    c_u = VT @ ug_w[0, :H] + ug_w[0, H:]
    c_m = VT @ sm_w[0]
    logits = x @ np.stack([c_f, c_u, c_m], axis=1)       # [B, 3]
    sig = 1.0 / (1.0 + np.exp(-(logits + np.array([fg_b[0], ug_b[0], sm_b[0]]))))
    fmean = float(sig[:, 0].mean())
    umean = float(sig[:, 1].mean())
    mix = sig[:, 2].astype(np.float32)                   # [B]

    xb = x.astype(bf16)
    x8 = np.clip(x, -240, 240).astype(fp8)
    mixs = (mix * OUT_SCALE).astype(np.float32)
    in_maps = []
    for i in range(NCORES):
        s = slice(i * BL, (i + 1) * BL)
        xTi = np.ascontiguousarray(xb[s].T)              # [D, BL]
        parts = []
        for j, g in enumerate(KGROUPS):
            n = len(g)
            xp = xTi[g[0] * P:(g[0] + n) * P].reshape(n, P, BL)
            parts.append(xp.transpose(1, 0, 2).reshape(P, n * BL))
            parts.append(we_parts[j])
        wx = np.ascontiguousarray(np.concatenate(parts, axis=1))
        in_maps.append({
            "wx": wx,
            "xn": np.ascontiguousarray(xb[s]),
            "x8": np.ascontiguousarray(x8[s].T).reshape(D // 2, 2 * BL),
            "m8": m8,
            "mx": np.ascontiguousarray(mixs[s].reshape(NB, P).T),
        })
    return in_maps, M, fmean, umean


def _finish(results, M, fmean, umean):
    out = np.concatenate([results[i]["out"] for i in range(NCORES)], axis=0)
    out *= 1.0 / OUT_SCALE
    delta = results[0]["dp"].astype(np.float64)
    for i in range(1, NCORES):
        delta += results[i]["dp"]
    delta_mean = (delta / B).astype(np.float32)
    new_M = fmean * M + umean * 0.1 * delta_mean
    return out.astype(np.float32), new_M.astype(np.float32)


def _run(inputs, trace=False, trace_kwargs=None):
    from concourse.bass_utils import run_bass_kernel_spmd

    nc = _build()
    in_maps, M, fmean, umean = _prepare(inputs)
    res = run_bass_kernel_spmd(nc, in_maps, core_ids=list(range(NCORES)),
                               trace=trace, **(trace_kwargs or {}))
    return _finish(res.results, M, fmean, umean), res


def kernel(**inputs):
    (out, new_M), _ = _run(inputs)
    return out, new_M
